# revision 13
# baseline (speedup 1.0000x reference)
"""CentroidAttention Trainium2 kernel (8 NeuronCores, SPMD data-parallel over batch).

Reference computation (per problem):
    centers = segment_mean(features, labels, C=1000)       # [C, F]
    q = features @ Wq; k = centers @ Wk; v = centers @ Wv  # [B,A],[C,A],[C,A]
    P = softmax(q @ k.T / sqrt(A))                         # [B, C]
    attn = P @ v @ Wproj + bproj                           # [B, F]
    out = concat([features, attn], -1)                     # [B, 2F]

Sharding: batch B=16384 split 8 ways (2048 rows/core). Each core computes
partial segment sums+counts (as a one-hot matmul, transposed layout
sums.T [F, C]), AllReduce's them, then runs the attention pipeline on its
own batch shard. Weights are replicated.

Precision strategy: the v-path (segsum, vU, PV, out) stays fp16 -- fp8
there pushes the final error past tolerance. The logit path (q, kU, S)
runs in fp8e4 with DoubleRow perf mode (2 K-tiles per instruction,
~1.44x PE throughput); softmax absorbs the small logit noise. Scale
folding keeps everything in e4m3's happy range:
  wq8 = Q8(64*Wq), featT8 = Q8(feat.T)     -> q psum = 64*qT
  qT8 = Q8(q psum / 16) = 4*qT
  wk8 = Q8(64*Wk), sums8 = Q8(sums.T)      -> k psum = 64*kU
  kT8 = Q8(k psum / 16) = 4*kU
  S psum = kT8.T @ qT8 = 16*S_true         -> exp scale = SCALE*rc/16

Softmax denominator and segment counts are partition reductions; instead
of burning PE cycles on ones-matmuls over the full moving dim, DVE
tree-adds fold the tiles first and a single tiny ones-matmul finishes
the partition reduction.

Device layout (all matmuls are out = lhsT.T @ rhs, K on partitions):
  - sums.T [F, C]   <- lhsT = feat chunk [B,F-chunk], rhs = onehot [B, C]
  - feat.T [F, B]   <- PE transposes fused in the segsum pass (same lhsT)
  - q.T   [A, B]    <- fp8 DoubleRow, lhsT = wq8 pairs, rhs = featT8 pairs
  - kU.T  [A, C]    <- fp8 DoubleRow, lhsT = wk8 pairs, rhs = sums8 pairs
  - vU    [C, A]    <- lhsT = sums.T (fp16), rhs = Wv (fp16)
  - S.T   [C, B]    <- fp8 DoubleRow, lhsT = kT8 pairs, rhs = qT8 pairs
  - exp: ACT Exp with per-partition scale = attn_scale * recip_counts[c]/16
  - v = vU * recip_counts[c] applied on PSUM evict (folds the v-path division)
  - denom [1, B]    <- DVE tree-add of expS.T tiles + one ones-matmul
  - attnU.T [A, B]  <- lhsT = v [C, A], rhs = expS.T   (unnormalized, fp16)
  - outU [B, F]     <- lhsT = attnU.T, rhs = Wproj, plus K=1 row
                       (lhsT=denom-row, rhs=bproj) so bias lands pre-normalized
  - out = outU * recip_denom[b]  (per-partition scale on final evict)

Classes padded 1000 -> 1024 (zero one-hot columns); the padded expS.T rows are
memset to 0 so they contribute nothing to denom or PV.
"""

import numpy as np

import concourse.bass as bass
import concourse.bacc as bacc
import concourse.mybir as mybir
import concourse.tile as tile
from concourse.bass_utils import run_bass_kernel_spmd
from concourse.masks import make_identity

P = 128
B_LOCAL = 2048          # batch rows per core
F = 1024                # feature dim
A = 512                 # attention dim
C = 1000                # num classes
CP = 1024               # classes padded to a multiple of 512
NB = B_LOCAL // P       # 16 batch chunks
NF = F // P             # 8 feature chunks
NA = A // P             # 4 attn-dim chunks
NCC = CP // P           # 8 class chunks
NN = B_LOCAL // 512     # 4 moving-operand chunks over local batch
N_CORES = 8
SCALE = float(A) ** -0.5

F32 = mybir.dt.float32
F16 = mybir.dt.float16
F8 = mybir.dt.float8e4
DR = mybir.MatmulPerfMode.DoubleRow

W8_SCALE = 64.0         # Wq/Wk prescale before fp8 quant
EV_SCALE = 1.0 / 16.0   # q/k PSUM evict scale (keeps fp8 values ~N(0,3))
S_PSUM_SCALE = 16.0     # resulting S psum = 16 * true logits


def _emit(tc, collective=True, io=None):
    nc = tc.nc
    if io is None:
        io = _declare_io(nc)
    (feat_dram, lab_dram, wq_dram, wk_dram, wv_dram, wp_dram, bp_dram,
     out_dram) = io

    from contextlib import ExitStack

    with ExitStack() as ctx:
        consts = ctx.enter_context(tc.tile_pool(name="consts", bufs=1))
        stage = ctx.enter_context(tc.tile_pool(name="stage", bufs=1))
        featn_pool = ctx.enter_context(tc.tile_pool(name="featn", bufs=1))
        p1024 = ctx.enter_context(tc.tile_pool(name="p1024", bufs=1))
        t2048 = ctx.enter_context(tc.tile_pool(name="t2048", bufs=1))
        wpool = ctx.enter_context(tc.tile_pool(name="wpool", bufs=1))
        vpool = ctx.enter_context(tc.tile_pool(name="vpool", bufs=1))
        f8pool = ctx.enter_context(tc.tile_pool(name="f8pool", bufs=1))
        dram = ctx.enter_context(tc.tile_pool(name="dram", bufs=1, space="DRAM"))

        STAGE_BUFS = 2
        C1024_BUFS = 16
        T2048_BUFS = 11
        pf16 = ctx.enter_context(tc.tile_pool(name="pf16", bufs=1))

        def stage_tile(name):
            return stage.tile([P, 1024], F32, name=name, tag="stage", bufs=STAGE_BUFS)

        def c1024_tile(name):
            return p1024.tile([P, CP], F16, name=name, tag="c1024", bufs=C1024_BUFS)

        def t2048_tile(name):
            return t2048.tile([P, B_LOCAL], F16, name=name, tag="t2048",
                              bufs=T2048_BUFS)

        # ---- constants ----
        identity = consts.tile([P, P], F16, name="identity")
        make_identity(nc, identity)
        ones_col = consts.tile([P, 1], F16, name="ones_col")
        nc.gpsimd.memset(ones_col, 1.0)
        ones_row = consts.tile([1, P], F16, name="ones_row")
        nc.gpsimd.memset(ones_row, 1.0)
        ones8 = consts.tile([P, 1], F8, name="ones8")
        nc.gpsimd.memset(ones8, 1.0)
        iota_g = consts.tile([P, CP], F16, name="iota_g")
        nc.gpsimd.iota(iota_g, pattern=[[1, CP]], base=0, channel_multiplier=0,
                       allow_small_or_imprecise_dtypes=True)
        # labels funnel through DVE so the one-hot tensor_scalar
        # (pointer-scalar variant, single sync-wait slot) only waits on
        # the gpsimd iota
        iota = iota_g
        labels_ld = consts.tile([P, NB], F32, name="labels_ld")
        nc.sync.dma_start(labels_ld, lab_dram)
        labels_sb = consts.tile([P, NB], F32, name="labels_sb")
        nc.vector.tensor_copy(labels_sb, labels_ld)
        # warm the ACT Exp table during the load phase so the table DMA
        # doesn't land in the middle of the softmax
        exp_warm = consts.tile([P, 1], F32, name="exp_warm")
        nc.scalar.activation(exp_warm, labels_sb[:, 0:1],
                             mybir.ActivationFunctionType.Exp,
                             bias=0.0, scale=0.0)

        # ---- collective bounce buffers. counts ride their own tiny f32
        # reduce launched during the segsum; the two sums halves go as fp16
        # (matching the fp16 compute dtype) so each collective moves 1 MB ----
        QTR = 2 * P  # one j-pair (256 rows of sums.T) per collective
        bcnt_in = dram.tile([P, NCC], F16, name="bcnt_in")
        bcnt_out = dram.tile([P, NCC], F16, name="bcnt_out",
                             addr_space="Shared")
        bnc_in, bnc_out = [], []
        for q in range(4):
            bnc_in.append(dram.tile([QTR, CP], F16, name=f"bnc_in{q}"))
            bnc_out.append(dram.tile([QTR, CP], F16, name=f"bnc_out{q}",
                                     addr_space="Shared"))

        # ---- phase 0: load features (cast fp16) and build one-hot ----
        feats = []
        for k in range(NB):
            st = stage_tile(f"fst{k}")
            nc.sync.dma_start(st, feat_dram[k * P:(k + 1) * P, :])
            fb = featn_pool.tile([P, F], F16, name=f"featN{k}")
            # ACT does the cast: DVE is saturated building one-hots during
            # the feature-load chase
            nc.scalar.copy(fb, st)
            feats.append(fb)
        onehots = []
        cnt_acc = consts.tile([P, CP], F8, name="cnt_acc")
        for k in range(NB):
            oh = c1024_tile(f"onehot{k}")
            nc.vector.tensor_scalar(oh, iota, labels_sb[:, k:k + 1], None,
                                    mybir.AluOpType.is_equal)
            onehots.append(oh)
            # fold counts on DVE as the one-hots appear (values stay exact
            # in fp16: per-core counts <= 2048)
            with nc.allow_low_precision(reason="counts <= 2048 exact"):
                if k == 1:
                    nc.vector.tensor_tensor(cnt_acc, onehots[0], oh,
                                            mybir.AluOpType.add)
                elif k > 1:
                    nc.vector.tensor_tensor(cnt_acc, cnt_acc, oh,
                                            mybir.AluOpType.add)

        # ---- phase A: counts partition-reduced straight into transposed
        # [class-partition, chunk] layout: lhsT = cnt_acc chunk, rhs = ones
        # (N=1 moving) -> countsT[:, c]. Collective reduces the tiny [P, NCC]
        # tile; no PE transposes needed downstream ----
        with tc.tile_pool(name="pcnt", bufs=1, space="PSUM") as pcnt:
            cps = pcnt.tile([P, NCC], F32, name="counts_ps")
            for c in range(NCC):
                nc.tensor.matmul(cps[:, c:c + 1],
                                 lhsT=cnt_acc[:, c * P:(c + 1) * P],
                                 rhs=ones8,
                                 start=True, stop=True)
            cnt_sb = consts.tile([P, NCC], F16, name="cnt_sb")
            nc.vector.tensor_copy(cnt_sb, cps)
            nc.sync.dma_start(bcnt_in, cnt_sb)
        # counts reduce in flight while the segment sums still compute
        if collective:
            nc.gpsimd.collective_compute(
                "AllReduce", mybir.AluOpType.add,
                replica_groups=[list(range(N_CORES))],
                ins=[bcnt_in.opt()], outs=[bcnt_out.opt()],
            )
        else:
            nc.sync.dma_start(bcnt_out, bcnt_in)

        # ---- fp8 pair tiles (DoubleRow operands) ----
        featT8 = [f8pool.tile([P, 2, B_LOCAL], F8, name=f"featT8_{q}")
                  for q in range(NF // 2)]
        wq8 = [f8pool.tile([P, 2, A], F8, name=f"wq8_{q}")
               for q in range(NF // 2)]
        wk8 = [f8pool.tile([P, 2, A], F8, name=f"wk8_{q}")
               for q in range(NF // 2)]
        sums8 = [f8pool.tile([P, 2, CP], F8, name=f"sums8_{q}")
                 for q in range(NF // 2)]
        qT8 = [f8pool.tile([P, 2, B_LOCAL], F8, name=f"qT8_{p}")
               for p in range(NA // 2)]
        kT8 = [f8pool.tile([P, 2, CP], F8, name=f"kT8_{p}")
               for p in range(NA // 2)]

        # ---- phase B: segment sums (transposed) + feat.T via fused PE
        # transpose. F-chunks processed in pairs so the PE has ~2x work per
        # arriving feature chunk during the initial DMA chase.
        with tc.tile_pool(name="pseg", bufs=1, space="PSUM") as pseg:
            for jp in range(0, NF, 2):
                sps_p, ftA_p, ftB_p = {}, {}, {}
                for j in (jp, jp + 1):
                    sps_p[j] = pseg.tile([P, CP], F32, name=f"sums{j}",
                                         tag="sums", bufs=2)
                    ftA_p[j] = pseg.tile([P, F], F16, name=f"ftA{j}",
                                         tag="ftA", bufs=2)
                    ftB_p[j] = pseg.tile([P, F], F16, name=f"ftB{j}",
                                         tag="ftB", bufs=2)
                for k in range(NB):
                    for j in (jp, jp + 1):
                        lhsT = feats[k][:, j * P:(j + 1) * P]
                        for h in range(2):
                            nc.tensor.matmul(
                                sps_p[j][:, h * 512:(h + 1) * 512],
                                lhsT=lhsT,
                                rhs=onehots[k][:, h * 512:(h + 1) * 512],
                                start=(k == 0), stop=(k == NB - 1))
                        ft = ftA_p[j] if k < 8 else ftB_p[j]
                        nc.tensor.transpose(ft[:, (k % 8) * P:(k % 8 + 1) * P],
                                            lhsT, identity)
                for j in (jp, jp + 1):
                    # feat.T lands directly in the fp8 DoubleRow pair tile
                    nc.vector.tensor_copy(
                        featT8[jp // 2][:, j - jp, 0:F], ftA_p[j])
                    nc.vector.tensor_copy(
                        featT8[jp // 2][:, j - jp, F:2 * F], ftB_p[j])
                    sums_sb = pf16.tile([P, CP], F16, name=f"sums_f16_{j}",
                                        tag="sf16", bufs=1)
                    nc.vector.tensor_copy(sums_sb, sps_p[j])
                    nc.sync.dma_start(
                        bnc_in[jp // 2][(j - jp) * P:(j - jp + 1) * P, :],
                        sums_sb)
                # reduce this pair while the next pair computes
                if collective:
                    nc.gpsimd.collective_compute(
                        "AllReduce", mybir.AluOpType.add,
                        replica_groups=[list(range(N_CORES))],
                        ins=[bnc_in[jp // 2].opt()],
                        outs=[bnc_out[jp // 2].opt()],
                    )
                else:
                    nc.sync.dma_start(bnc_out[jp // 2], bnc_in[jp // 2])

        # ---- weights: load + cast (overlaps the collective). Wq/Wk go to
        # fp8 pair tiles prescaled by 64 so e4m3 sees ~N(0,1.3) values ----
        for nm, src, dst in (("wq", wq_dram, wq8), ("wk", wk_dram, wk8)):
            for j in range(NF):
                st = stage_tile(f"{nm}st{j}")
                nc.sync.dma_start(st[:, 0:A], src[j * P:(j + 1) * P, :])
                nc.vector.tensor_scalar_mul(dst[j // 2][:, j % 2, :],
                                            st[:, 0:A], W8_SCALE)
        wvb = []
        for j in range(NF):
            st = stage_tile(f"wvst{j}")
            nc.sync.dma_start(st[:, 0:A], wv_dram[j * P:(j + 1) * P, :])
            wb = wpool.tile([P, A], F16, name=f"wvb{j}")
            nc.vector.tensor_copy(wb, st[:, 0:A])
            wvb.append(wb)
        wpb = []
        for a in range(NA):
            st = stage_tile(f"wpst{a}")
            nc.sync.dma_start(st, wp_dram[a * P:(a + 1) * P, :])
            wb = wpool.tile([P, F], F16, name=f"wpb{a}")
            nc.vector.tensor_copy(wb, st)
            wpb.append(wb)
        bst = stage_tile("bst")
        nc.sync.dma_start(bst[0:1, :], bp_dram)
        bprojb = wpool.tile([1, F], F16, name="bprojb")
        nc.vector.tensor_copy(bprojb, bst[0:1, :])

        # ---- q.T = Wq.T @ feat.T, fp8 DoubleRow (PE busy during the
        # collective). q psum = 64*qT; evict at 1/16 -> qT8 = 4*qT ----
        with tc.tile_pool(name="pq", bufs=1, space="PSUM") as pq:
            for a in range(NA):
                for nh in range(2):
                    qps = pq.tile([P, F], F32, name=f"qps{a}_{nh}",
                                  tag="q", bufs=4)
                    for n in range(2):
                        for jp in range(NF // 2):
                            nc.tensor.matmul(
                                qps[:, n * 512:(n + 1) * 512],
                                lhsT=wq8[jp][:, :, a * P:(a + 1) * P],
                                rhs=featT8[jp][:, :, (nh * 2 + n) * 512:
                                               (nh * 2 + n + 1) * 512],
                                start=(jp == 0), stop=(jp == NF // 2 - 1),
                                perf_mode=DR)
                    nc.scalar.activation(
                        qT8[a // 2][:, a % 2, nh * F:(nh + 1) * F], qps,
                        mybir.ActivationFunctionType.Copy,
                        bias=0.0, scale=EV_SCALE)

        # ---- read back reduced sums + counts (fp16 lands matmul-ready;
        # fp8 pair copies feed the kU DoubleRow) ----
        sumsb = []
        for j in range(NF):
            sb = c1024_tile(f"sumsb{j}")
            nc.sync.dma_start(sb, bnc_out[j // 2][(j % 2) * P:(j % 2 + 1) * P, :])
            nc.vector.tensor_copy(sums8[j // 2][:, j % 2, :], sb)
            sumsb.append(sb)
        counts_sb = consts.tile([P, NCC], F16, name="counts_sb")
        nc.sync.dma_start(counts_sb, bcnt_out)

        vbs = []
        with tc.tile_pool(name="pkv", bufs=1, space="PSUM") as pkv:
            # recip_counts already in [C-chunk(partition), chunk-idx] layout
            cnt_m = consts.tile([P, NCC], F16, name="cnt_m")
            nc.vector.tensor_scalar_max(cnt_m, counts_sb, 1.0)
            recip_cols = consts.tile([P, NCC], F32, name="recip_cols")
            with nc.allow_low_precision(reason="recip counts, rel 5e-4"):
                nc.vector.reciprocal(recip_cols, cnt_m)
            exp_scale = consts.tile([P, NCC], F32, name="exp_scale")
            nc.vector.tensor_scalar_mul(exp_scale, recip_cols,
                                        SCALE / S_PSUM_SCALE)

            # kU.T [A, C] fp8 DoubleRow; counts division folded into the
            # exp scale later. k psum = 64*kU; evict 1/16 -> kT8 = 4*kU
            for a in range(NA):
                kps = pkv.tile([P, CP], F32, name=f"kps{a}", tag="k", bufs=2)
                for h in range(2):
                    for jp in range(NF // 2):
                        nc.tensor.matmul(
                            kps[:, h * 512:(h + 1) * 512],
                            lhsT=wk8[jp][:, :, a * P:(a + 1) * P],
                            rhs=sums8[jp][:, :, h * 512:(h + 1) * 512],
                            start=(jp == 0), stop=(jp == NF // 2 - 1),
                            perf_mode=DR)
                nc.scalar.activation(kT8[a // 2][:, a % 2, :], kps,
                                     mybir.ActivationFunctionType.Copy,
                                     bias=0.0, scale=EV_SCALE)

            # v [C, A] = (sums.T).T @ Wv (fp16), scaled by recip_counts
            for c in range(NCC):
                vps = pkv.tile([P, A], F32, name=f"vps{c}", tag="v", bufs=2)
                for j in range(NF):
                    nc.tensor.matmul(vps,
                                     lhsT=sumsb[j][:, c * P:(c + 1) * P],
                                     rhs=wvb[j],
                                     start=(j == 0), stop=(j == NF - 1))
                vb = vpool.tile([P, A], F16, name=f"vb{c}")
                nc.scalar.activation(vb, vps,
                                     mybir.ActivationFunctionType.Copy,
                                     bias=0.0, scale=recip_cols[:, c:c + 1])
                vbs.append(vb)

        # ---- S.T [C, B] fp8 DoubleRow and exp (centers division + fp8
        # scale factor folded into the exp scale). DVE tree-folds the
        # softmax denominator as the expS.T tiles appear ----
        dacc = consts.tile([P, B_LOCAL], F16, name="dacc")
        expSTs = []
        with tc.tile_pool(name="pst", bufs=1, space="PSUM") as pst:
            for c in range(NCC):
                est = t2048_tile(f"expST{c}")
                rows = (C - c * P) if c == NCC - 1 else P
                if rows < P:
                    # zero the padded class rows; exp overwrites the valid ones
                    nc.vector.memset(est, 0.0)
                for nh in range(2):
                    sps = pst.tile([P, F], F32, name=f"stps{c}_{nh}",
                                   tag="st", bufs=4)
                    for n in range(2):
                        for ap in range(NA // 2):
                            nc.tensor.matmul(
                                sps[:, n * 512:(n + 1) * 512],
                                lhsT=kT8[ap][:, :, c * P:(c + 1) * P],
                                rhs=qT8[ap][:, :, (nh * 2 + n) * 512:
                                            (nh * 2 + n + 1) * 512],
                                start=(ap == 0), stop=(ap == NA // 2 - 1),
                                perf_mode=DR)
                    nc.scalar.activation(est[0:rows, nh * F:(nh + 1) * F],
                                         sps[0:rows, :],
                                         mybir.ActivationFunctionType.Exp,
                                         bias=0.0,
                                         scale=exp_scale[0:rows, c:c + 1])
                expSTs.append(est)
                with nc.allow_low_precision(
                        reason="denom partials; f32 partition-reduce follows"):
                    if c == 1:
                        nc.vector.tensor_tensor(dacc, expSTs[0], est,
                                                mybir.AluOpType.add)
                    elif c > 1:
                        nc.vector.tensor_tensor(dacc, dacc, est,
                                                mybir.AluOpType.add)

        # ---- attnU.T [A, B] = v.T @ expS.T (unnormalized, fp16); the
        # softmax denominator needs only one tiny ones-matmul on dacc ----
        recipD_cols = consts.tile([P, NB], F32, name="recipD_cols")
        attnTs = []
        with tc.tile_pool(name="ppv", bufs=1, space="PSUM") as ppv:
            dps = ppv.tile([P, NB], F32, name="dps")
            for t in range(NB):
                nc.tensor.matmul(dps[:, t:t + 1],
                                 lhsT=dacc[:, t * P:(t + 1) * P],
                                 rhs=ones_col,
                                 start=True, stop=True)
            with nc.allow_low_precision(reason="recip denom, rel err 5e-4"):
                nc.vector.reciprocal(recipD_cols, dps)
            for a in range(NA):
                at = t2048_tile(f"attnT{a}")
                for nh in range(2):
                    aps = ppv.tile([P, F], F32, name=f"aps{a}_{nh}",
                                   tag="av", bufs=2)
                    for c in range(NCC):
                        for n in range(2):
                            nc.tensor.matmul(
                                aps[:, n * 512:(n + 1) * 512],
                                lhsT=vbs[c][:, a * P:(a + 1) * P],
                                rhs=expSTs[c][:, (nh * 2 + n) * 512:
                                              (nh * 2 + n + 1) * 512],
                                start=(c == 0), stop=(c == NCC - 1))
                    evict = nc.vector.tensor_copy if nh == 0 else nc.scalar.copy
                    evict(at[:, nh * F:(nh + 1) * F], aps)
                attnTs.append(at)

        # ---- out = (attnU.T.T @ Wproj) * recip_denom + bproj ----
        with tc.tile_pool(name="po", bufs=1, space="PSUM") as po:
            # bproj broadcast to all partitions (bias applied post-normalize)
            bpb_ps = po.tile([P, F], F32, name="bpb_ps")
            for h in range(2):
                nc.tensor.matmul(bpb_ps[:, h * 512:(h + 1) * 512],
                                 lhsT=ones_row,
                                 rhs=bprojb[:, h * 512:(h + 1) * 512],
                                 start=True, stop=True)
            bpb_sb = consts.tile([P, F], F16, name="bpb_sb")
            nc.vector.tensor_copy(bpb_sb, bpb_ps)
            for t in range(NB):
                ops = po.tile([P, F], F32, name=f"ops{t}", tag="o", bufs=2)
                for a in range(NA):
                    for h in range(2):
                        nc.tensor.matmul(ops[:, h * 512:(h + 1) * 512],
                                         lhsT=attnTs[a][:, t * P:(t + 1) * P],
                                         rhs=wpb[a][:, h * 512:(h + 1) * 512],
                                         start=(a == 0), stop=(a == NA - 1))
                osb = stage_tile(f"osb{t}")
                nc.vector.scalar_tensor_tensor(
                    osb, ops, recipD_cols[:, t:t + 1], bpb_sb,
                    op0=mybir.AluOpType.mult, op1=mybir.AluOpType.add)
                nc.sync.dma_start(out_dram[t * P:(t + 1) * P, :], osb)


def _declare_io(nc):
    return (
        nc.dram_tensor("features", [B_LOCAL, F], F32, kind="ExternalInput")[:],
        nc.dram_tensor("labels_f32", [P, NB], F32, kind="ExternalInput")[:],
        nc.dram_tensor("Wq", [F, A], F32, kind="ExternalInput")[:],
        nc.dram_tensor("Wk", [F, A], F32, kind="ExternalInput")[:],
        nc.dram_tensor("Wv", [F, A], F32, kind="ExternalInput")[:],
        nc.dram_tensor("Wproj", [A, F], F32, kind="ExternalInput")[:],
        nc.dram_tensor("bproj", [1, F], F32, kind="ExternalInput")[:],
        nc.dram_tensor("out", [B_LOCAL, F], F32, kind="ExternalOutput")[:],
    )


_BUILT = {}


def _get_nc(collective=True, reps=1):
    key = (collective, reps)
    if key not in _BUILT:
        nc = bacc.Bacc("TRN2", target_bir_lowering=False, debug=False,
                       num_devices=N_CORES)
        with tile.TileContext(nc) as tc:
            io = _declare_io(nc)
            for r in range(reps):
                if r:
                    tc.strict_bb_all_engine_barrier()
                _emit(tc, collective=collective, io=io)
        nc.compile()
        _BUILT[key] = nc
    return _BUILT[key]


def _make_in_maps(inputs):
    features = np.ascontiguousarray(np.asarray(inputs["features"],
                                               dtype=np.float32))
    labels = np.ascontiguousarray(np.asarray(inputs["labels"])).astype(np.int64)
    Wq = np.ascontiguousarray(np.asarray(inputs["Wq"], dtype=np.float32))
    Wk = np.ascontiguousarray(np.asarray(inputs["Wk"], dtype=np.float32))
    Wv = np.ascontiguousarray(np.asarray(inputs["Wv"], dtype=np.float32))
    Wproj = np.ascontiguousarray(np.asarray(inputs["Wproj"], dtype=np.float32))
    bproj = np.ascontiguousarray(
        np.asarray(inputs["bproj"], dtype=np.float32)).reshape(1, F)

    in_maps = []
    for cix in range(N_CORES):
        fl = features[cix * B_LOCAL:(cix + 1) * B_LOCAL]
        ll = labels[cix * B_LOCAL:(cix + 1) * B_LOCAL]
        lab2d = np.ascontiguousarray(
            ll.astype(np.float32).reshape(NB, P).T)
        in_maps.append({
            "features": fl,
            "labels_f32": lab2d,
            "Wq": Wq, "Wk": Wk, "Wv": Wv, "Wproj": Wproj, "bproj": bproj,
        })
    return in_maps


def _assemble(inputs, results):
    features = np.asarray(inputs["features"], dtype=np.float32)
    out = np.empty((N_CORES * B_LOCAL, 2 * F), np.float32)
    out[:, :F] = features
    for cix in range(N_CORES):
        out[cix * B_LOCAL:(cix + 1) * B_LOCAL, F:] = results[cix]["out"]
    return out


def _run(inputs, **run_kwargs):
    nc = _get_nc()
    in_maps = _make_in_maps(inputs)
    res = run_bass_kernel_spmd(nc, in_maps, list(range(N_CORES)), **run_kwargs)
    return _assemble(inputs, res.results), res


def kernel(**inputs):
    out, _ = _run(inputs)
    return out
